# revision 58
# baseline (speedup 1.0000x reference)
"""Trainium2 Bass kernel for nn_Encoder_90469191122997 (gnn_message_passing).

Data-parallel over batch B=8: core b owns batch b end-to-end.

v3: 16-bit traffic + SBUF residency + phase-major layout + no barriers.

x is sent to the device as bf16 (host cast) in PHASE-MAJOR layout
([TC, dh, dw, h0, w0], the four 2x2-nearest phases separated), y returns
bf16 phase-major (host upcast + re-interleave).  The 2e-2 rel-err gate
leaves ~4x margin.

Per core x_b = [T*C, HW] bf16 = 28.9 MB.  Chunks 0..6 (128 rows each)
stay resident in SBUF between the pooling pass and the residual pass;
chunks 7, 8 stream through two quarter-width buffers and are re-read in
pass 2 (re-read DMAs issued before the GCN so DMA never idles).  HBM
traffic/core: 28.9 (read) + 6.4 (re-read) + 28.9 (write) = 64 MB vs
173 MB for the fp32 two-pass baseline.

Phase-major makes the 2x2 box-sum three full-width step-1 bf16 adds
(DVE 2x packed mode) and the nearest-upsample residual adds pure step-1
adds against a PSUM->SBUF copy of the residual (also 2x).  Pooling
contracts the box-summed transpose (PE, 112-col grid, no tail) against
pre-transposed masks; the 18-node GCN runs on-chip in fp32 PSUM with
bf16 operands.  All tile pools live at one scope: no per-rep drain
barriers; x2 and m56 timeshare one buffer (m56 re-loaded per rep).
"""

import numpy as np
import ml_dtypes

import concourse.bass as bass
import concourse.mybir as mybir
import concourse.tile as tile
from concourse.masks import make_identity

T, B, C, H, W = 6, 8, 192, 112, 112
K = 3
H0, W0 = 56, 56
HW = H * W            # 12544
HW0 = H0 * W0         # 3136
QW = HW0 // 4         # 784, quarter width at 56-res (phase-plane cols)
N = T * K             # 18
CH = 96               # c half
NJ = 28               # pooling blocks per chunk (112-col grid, no tail)
JW = 112              # pooling block width
NR = 8                # residual blocks per chunk
RW = HW0 // NR        # 392, residual block width at 56-res
NCH = T * C // 128    # 9 row-chunks of 128 (t,c) rows each
NSTASH = 8            # chunks 0..7 resident in SBUF; 8 streamed+re-read

BF = mybir.dt.bfloat16
F32 = mybir.dt.float32
BF_NP = ml_dtypes.bfloat16


def _spans(r):
    """(t, lo, hi, clo): rows [lo,hi) of chunk r belong to t, starting at
    channel clo.  Chunk boundaries hit t-edges only at offsets 0/64."""
    out = []
    for t in range(T):
        lo = max(128 * r, C * t)
        hi = min(128 * r + 128, C * (t + 1))
        if lo < hi:
            out.append((t, lo - 128 * r, hi - 128 * r, lo - C * t))
    return out


_LAST_CHUNK = {t: (C * (t + 1) - 1) // 128 for t in range(T)}

_MAX_WAITS = 1


def _split_multi_waits(nc):
    """This container's walrus rejects >1 sem wait per instruction ("Too many
    sync wait commands").  Move extra waits onto same-engine NoOps inserted
    immediately before the instruction (per-engine program order preserved)."""
    for bb in nc.main_func.blocks:
        insts = list(bb.instructions)
        if not any(
            i.sync_info and i.sync_info.on_wait
            and len(i.sync_info.on_wait) > _MAX_WAITS
            for i in insts
        ):
            continue
        new = []
        for inst in insts:
            si = inst.sync_info
            if si and si.on_wait and len(si.on_wait) > _MAX_WAITS:
                extra = list(si.on_wait[_MAX_WAITS:])
                del si.on_wait[_MAX_WAITS:]
                while extra:
                    chunk, extra = extra[:_MAX_WAITS], extra[_MAX_WAITS:]
                    nop = mybir.InstNoOp(
                        name=nc.get_next_instruction_name(),
                        engine=inst.engine,
                        bass_nofuse=True,
                        sync_info=mybir.SyncInfo(on_wait=chunk, on_update=[]),
                    )
                    nc.register_instruction(nop, overwrite=True)
                    new.append(nop)
            new.append(inst)
        bb.instructions = new


_orig_drain_and_barrier = tile.TileContext._drain_and_barrier


def _patched_drain_and_barrier(self, tick_clock, wait_clock):
    _orig_drain_and_barrier(self, tick_clock, wait_clock)
    _split_multi_waits(self.nc)


tile.TileContext._drain_and_barrier = _patched_drain_and_barrier


KNOBS = dict(
    copy_eng='scalar',     # engine for PSUM->SBUF transpose-tile copies
    add_eng='vector',      # engine for pass-2 residual adds
    rcopy_eng='scalar',    # engine for pass-2 res PSUM->SBUF copies
    store_eng='scalar',    # engine issuing y store DMAs
    x2T_bufs=2, tr_bufs=2, res_bufs=2, rsb_bufs=2, feat_bufs=3,
    pool_chunks=(),        # stash chunks whose box-sum runs on POOL
)

if __name__ != "__main__":
    import json as _json
    import os as _os
    _ov = _os.environ.get("KERNEL_KNOBS")
    if _ov:
        KNOBS.update(_json.loads(_ov))


def _copy(eng, dst, src):
    if hasattr(eng, 'tensor_copy'):
        eng.tensor_copy(dst, src)
    else:
        eng.copy(dst, src)


def build_nc(reps: int = 1) -> bass.Bass:
    # no dynamic DMAs anywhere: shrink the 16 KB/partition SWDGE scratch
    # to the minimum walrus accepts so an 8th x chunk fits in SBUF
    nc = bass.Bass(dynamic_dma_scratch_size=256)
    x = nc.dram_tensor("x", [T * C, HW], BF, kind="ExternalInput")
    m56 = nc.dram_tensor("m56", [N, HW0], BF, kind="ExternalInput")
    mTp = nc.dram_tensor("mTp", [JW, T * NJ * K], BF, kind="ExternalInput")
    wembT = nc.dram_tensor("wembT", [C, C], BF, kind="ExternalInput")
    wgcn = nc.dram_tensor("wgcn", [C, C], BF, kind="ExternalInput")
    bb = nc.dram_tensor("bb", [N, C], F32, kind="ExternalInput")
    y = nc.dram_tensor("y", [T * C, HW], BF, kind="ExternalOutput")

    copy_eng = getattr(nc, KNOBS['copy_eng'])
    add_eng = getattr(nc, KNOBS['add_eng'])
    rcopy_eng = getattr(nc, KNOBS['rcopy_eng'])
    store_eng = getattr(nc, KNOBS['store_eng'])

    with tile.TileContext(nc) as tc:
        with (
            tc.tile_pool(name="persist", bufs=1) as pp,
            tc.tile_pool(name="x2Tpool", bufs=KNOBS['x2T_bufs']) as x2Tpool,
            tc.tile_pool(name="smallsb", bufs=1) as ssb,
            tc.tile_pool(name="ressb", bufs=KNOBS['rsb_bufs']) as rsb,
            tc.tile_pool(name="lhsrp", bufs=2) as lhsrp,
            tc.tile_pool(name="trbps", bufs=KNOBS['tr_bufs'],
                         space="PSUM") as trbps,
            tc.tile_pool(name="featps", bufs=KNOBS['feat_bufs'],
                         space="PSUM") as fps,
            tc.tile_pool(name="ntps", bufs=1, space="PSUM") as ntps,
            tc.tile_pool(name="resps", bufs=KNOBS['res_bufs'],
                         space="PSUM") as rps,
        ):
            ident = pp.tile([128, 128], BF)
            make_identity(nc, ident)
            # only ever used as ident32[:N, :N] (GCN transposes)
            ident32 = pp.tile([32, 32], F32, tag="ident32")
            make_identity(nc, ident32)
            mTp_sb = pp.tile([JW, T * NJ * K], BF)
            nc.sync.dma_start(mTp_sb[:], mTp[:])
            wemb_h = []
            wgcn_h = []
            for hh in range(2):
                wt = pp.tile([CH, C], BF, tag=f"wemb{hh}")
                nc.sync.dma_start(wt[:], wembT[hh * CH:(hh + 1) * CH, :])
                wemb_h.append(wt)
                gt = pp.tile([CH, C], BF, tag=f"wgcn{hh}")
                nc.sync.dma_start(gt[:], wgcn[hh * CH:(hh + 1) * CH, :])
                wgcn_h.append(gt)
            bb_sb = pp.tile([N, C], F32)
            nc.sync.dma_start(bb_sb[:], bb[:])

            # resident x chunks, two quarter-width stream buffers, and the
            # x2 / m56 timeshared scratch
            st = [
                pp.tile([128, HW], BF, tag=f"stash{i}", name=f"stash{i}")
                for i in range(NSTASH)
            ]
            hb = pp.tile([128, HW // 4], BF, tag="hb", name="hb")
            # x2 box-sum scratch as two half-width buffers so chunk r+1's
            # first half overlaps chunk r's second-half transposes; in
            # pass 2 the same buffers hold the m56 mask halves (re-loaded
            # each rep)
            uh = [
                pp.tile([128, HW0 // 2], BF, tag=f"uh{i}", name=f"uh{i}")
                for i in range(2)
            ]


            for rep in range(reps):
                nodeT_h = [
                    pp.tile([CH, N], BF, tag=f"nodeT{hh}", name=f"nodeT{hh}")
                    for hh in range(2)
                ]
                outgb = pp.tile([N, C], BF, tag="outgb", name="outgb")

                # ---------------- pass 1: pooling ----------------
                feat_ps = {}

                def do_blocks(r, jj):
                    """x2T tile [112, 512] for 1-4 j-blocks of chunk r +
                    pooling matmuls.  Grouping amortizes the PSUM->SBUF
                    copy.  j's half decides which uh buffer holds its
                    box-sum."""
                    tr = trbps.tile([JW, 512], BF, tag="trb")
                    for i, j in enumerate(jj):
                        jl = j % (NJ // 2)
                        nc.tensor.transpose(
                            tr[:, 128 * i:128 * (i + 1)],
                            uh[j // (NJ // 2)][:, jl * JW:(jl + 1) * JW],
                            ident[:],
                        )
                    x2T = x2Tpool.tile([JW, 512], BF, tag="x2T")
                    w = 128 * len(jj)
                    _copy(copy_eng, x2T[:, :w], tr[:, :w])
                    for i, j in enumerate(jj):
                        for (t, lo, hi, clo) in _spans(r):
                            col = (t * NJ + j) * K
                            nc.tensor.matmul(
                                feat_ps[t][:, clo:clo + (hi - lo)],
                                mTp_sb[:, col:col + K],
                                x2T[:, 128 * i + lo:128 * i + hi],
                                start=(j == 0), stop=(j == NJ - 1),
                                skip_group_check=True,
                            )

                def boxsum(xq, dst):
                    """dst = sum of the 4 phase planes (step-1 bf16 adds,
                    DVE 2x packed mode)."""
                    nc.vector.tensor_add(dst, xq[:, 0, :], xq[:, 1, :])
                    nc.vector.tensor_add(dst, dst, xq[:, 2, :])
                    nc.vector.tensor_add(dst, dst, xq[:, 3, :])

                chunks_left = {
                    t: {r for r in range(NCH)
                        if any(s[0] == t for s in _spans(r))}
                    for t in range(T)
                }

                def close_feat(r):
                    for (t, lo, hi, clo) in _spans(r):
                        chunks_left[t].discard(r)
                        if chunks_left[t]:
                            continue
                        feat_sb = ssb.tile([K, C], BF, tag="feat_sb")
                        nc.scalar.mul(feat_sb[:], feat_ps.pop(t)[:], 1.0 / HW)
                        for hh in range(2):
                            ntr = ntps.tile([CH, K], BF, tag="ntr")
                            nc.tensor.transpose(
                                ntr[:],
                                feat_sb[:, hh * CH:(hh + 1) * CH],
                                ident[:K, :K],
                            )
                            nc.any.tensor_copy(
                                nodeT_h[hh][:, K * t:K * (t + 1)], ntr[:]
                            )

                def open_feat(r):
                    for (t, lo, hi, clo) in _spans(r):
                        if t not in feat_ps:
                            feat_ps[t] = fps.tile(
                                [K, C], F32, tag="feat_ps", name=f"featps{t}"
                            )

                for r in range(NSTASH):
                    buf = st[r]
                    nc.sync.dma_start(buf[:], x[128 * r:128 * (r + 1), :])
                    xq = buf.rearrange("p (q c) -> p q c", q=4)
                    open_feat(r)
                    # box-sum in two halves so PE transposes of the first
                    # half overlap DVE summing the second
                    for hf in range(2):
                        sl = slice(hf * (HW0 // 2), (hf + 1) * (HW0 // 2))
                        if r in KNOBS['pool_chunks']:
                            xv = xq[:, :, sl]
                            dst = uh[hf][:, :]
                            nc.gpsimd.tensor_add(dst, xv[:, 0, :], xv[:, 1, :])
                            nc.gpsimd.tensor_add(dst, dst, xv[:, 2, :])
                            nc.gpsimd.tensor_add(dst, dst, xv[:, 3, :])
                        else:
                            boxsum(xq[:, :, sl], uh[hf][:, :])
                        jh = list(range(hf * (NJ // 2), (hf + 1) * (NJ // 2)))
                        for i in range(0, NJ // 2, 4):
                            do_blocks(r, tuple(jh[i:i + 4]))
                    close_feat(r)

                # chunk 8 streams through the single quarter buffer; its
                # box-sums run on the otherwise-idle POOL engine so the
                # load chain never waits on DVE's in-order backlog.
                # Quarter qt holds phase-plane cols [qt*QW, (qt+1)*QW).
                def qslice(r, qt):
                    xv = x[128 * r:128 * (r + 1), :].rearrange(
                        "p (q c) -> p q c", q=4)
                    return xv[:, :, qt * QW:(qt + 1) * QW]

                for r in (8,):
                    open_feat(r)
                    for qt in range(4):
                        nc.sync.dma_start(
                            hb.rearrange("p (q c) -> p q c", q=4)[:],
                            qslice(r, qt),
                        )
                        dst = uh[qt // 2][:, (qt % 2) * QW:(qt % 2 + 1) * QW]
                        xqv = hb.rearrange("p (q c) -> p q c", q=4)
                        nc.gpsimd.tensor_add(dst, xqv[:, 0, :], xqv[:, 1, :])
                        nc.gpsimd.tensor_add(dst, dst, xqv[:, 2, :])
                        nc.gpsimd.tensor_add(dst, dst, xqv[:, 3, :])
                        # a quarter holds 7 j-blocks (3.5 pairs): pair within
                        # the quarter, odd block alone
                        jq = list(range(qt * (NJ // 4), (qt + 1) * (NJ // 4)))
                        for i in range(0, NJ // 4, 2):
                            do_blocks(r, tuple(jq[i:i + 2]))
                    close_feat(r)

                # pass-2 data for chunk 8 comes back via the same quarter
                # buffer.  The first re-read lands during the GCN; each
                # later one is issued as the previous quarter is consumed.
                # re-reads and the m56 loads ride the scalar DMA queue so
                # they never head-of-line-block the next rep's x loads on
                # the sync queue
                def reread(qt):
                    nc.scalar.dma_start(
                        hb.rearrange("p (q c) -> p q c", q=4)[:],
                        qslice(8, qt),
                    )

                reread(0)
                # the mask halves share the uh buffers with x2: load after
                # pass 1's last transposes
                for hf in range(2):
                    nc.scalar.dma_start(
                        uh[hf][:N, :],
                        m56[:, hf * (HW0 // 2):(hf + 1) * (HW0 // 2)],
                    )

                # ---------------- GCN on [18, 192] ----------------
                # no PSUM bank of its own: GCN tiles ride the res pool's
                # tag (same dtype, big enough, and that ring is idle here)
                def gtile():
                    return rps.tile([128, RW], F32, tag="res", name="g")

                adjL = gtile()
                for hh in range(2):
                    nc.tensor.matmul(
                        adjL[:N, :N], nodeT_h[hh][:], nodeT_h[hh][:],
                        start=(hh == 0), stop=(hh == 1),
                    )
                mx = ssb.tile([N, 1], F32, tag="mx")
                nc.vector.reduce_max(mx[:], adjL[:N, :N],
                                     axis=mybir.AxisListType.X)
                nmx = ssb.tile([N, 1], F32, tag="nmx")
                nc.vector.tensor_scalar_mul(nmx[:], mx[:], -1.0)
                e_sb = ssb.tile([N, N], F32, tag="e_sb")
                nc.scalar.activation(
                    e_sb[:], adjL[:N, :N], mybir.ActivationFunctionType.Exp,
                    bias=nmx[:], scale=1.0,
                )
                s_ = ssb.tile([N, 1], F32, tag="s_")
                nc.vector.reduce_sum(s_[:], e_sb[:], axis=mybir.AxisListType.X)
                r_ = ssb.tile([N, 1], F32, tag="r_")
                nc.vector.reciprocal(r_[:], s_[:])
                adj_f = ssb.tile([N, N], F32, tag="adj_f")
                nc.vector.tensor_scalar_mul(adj_f[:], e_sb[:], r_[:])

                aaa_ps = gtile()
                for hh in range(2):
                    nc.tensor.matmul(
                        aaa_ps[:N, :C], nodeT_h[hh][:], wemb_h[hh][:],
                        start=(hh == 0), stop=(hh == 1),
                    )
                aaa_f = ssb.tile([N, C], F32, tag="aaa_f")
                nc.scalar.copy(aaa_f[:], aaa_ps[:N, :C])
                aaaT_h = []
                for hh in range(2):
                    aT_ps = gtile()
                    nc.tensor.transpose(
                        aT_ps[:CH, :N], aaa_f[:, hh * CH:(hh + 1) * CH],
                        ident32[:N, :N],
                    )
                    aT = ssb.tile([CH, N], BF, tag=f"aaaT{hh}")
                    nc.scalar.copy(aT[:], aT_ps[:CH, :N])
                    aaaT_h.append(aT)
                supp_ps = gtile()
                for hh in range(2):
                    nc.tensor.matmul(
                        supp_ps[:N, :C], aaaT_h[hh][:], wgcn_h[hh][:],
                        start=(hh == 0), stop=(hh == 1),
                    )
                supp_b = ssb.tile([N, C], BF, tag="supp_b")
                nc.scalar.copy(supp_b[:], supp_ps[:N, :C])
                adjT_ps = gtile()
                nc.tensor.transpose(adjT_ps[:N, :N], adj_f[:],
                                    ident32[:N, :N])
                adjT_b = ssb.tile([N, N], BF, tag="adjT_b")
                nc.scalar.copy(adjT_b[:], adjT_ps[:N, :N])
                outg_ps = gtile()
                nc.tensor.matmul(
                    outg_ps[:N, :C], adjT_b[:], supp_b[:], start=True, stop=True
                )
                nc.vector.tensor_add(outgb[:], outg_ps[:N, :C], bb_sb[:])

                # ---------------- pass 2: residual ----------------
                def make_lhsr(r):
                    """[18, 128] tile: outg rows 3t:3t+3 in the column range
                    of each t-span, zeros elsewhere.  Small SBUF->SBUF DMAs
                    go on the store queue to stay clear of the re-reads."""
                    L = lhsrp.tile([N, 128], BF, tag="lhsr")
                    nc.vector.memset(L[:], 0.0)
                    for (t, lo, hi, clo) in _spans(r):
                        store_eng.dma_start(
                            L[K * t:K * (t + 1), lo:hi],
                            outgb[K * t:K * (t + 1), clo:clo + (hi - lo)],
                        )
                    return L

                def res_half(L, h):
                    """Residual for hw0 cols [h*HW0/2, (h+1)*HW0/2) ->
                    bf16 SBUF tile (copies feed the 2x-mode adds).  Masks
                    come from the uh buffer holding half h."""
                    rs = rsb.tile([128, HW0 // 2], BF, tag="rs")
                    for j in range(NR // 2):
                        res = rps.tile([128, RW], F32, tag="res")
                        nc.tensor.matmul(
                            res[:],
                            L[:],
                            uh[h][:N, j * RW:(j + 1) * RW],
                            start=True, stop=True,
                        )
                        _copy(rcopy_eng, rs[:, j * RW:(j + 1) * RW], res[:])
                    return rs

                # chunk 8's lhsr must survive the whole interleaved pass:
                # dedicated persistent tile instead of the 2-deep ring
                L8 = pp.tile([N, 128], BF, tag="lhsr8", name="lhsr8")
                nc.vector.memset(L8[:], 0.0)
                for (t, lo, hi, clo) in _spans(8):
                    store_eng.dma_start(
                        L8[K * t:K * (t + 1), lo:hi],
                        outgb[K * t:K * (t + 1), clo:clo + (hi - lo)],
                    )

                def do_quarter8(qt):
                    """One re-read quarter of chunk 8: 2 residual blocks,
                    4 plane adds, store, then kick off the next re-read."""
                    rs = rsb.tile([128, HW0 // 2], BF, tag="rs")
                    for j in range(2):
                        res = rps.tile([128, RW], F32, tag="res")
                        nc.tensor.matmul(
                            res[:],
                            L8[:],
                            uh[qt // 2][:N, ((qt % 2) * 2 + j) * RW:
                                        ((qt % 2) * 2 + j + 1) * RW],
                            start=True, stop=True,
                        )
                        _copy(rcopy_eng, rs[:, j * RW:(j + 1) * RW], res[:])
                    xq = hb.rearrange("p (q c) -> p q c", q=4)
                    for q in range(4):
                        add_eng.tensor_add(xq[:, q, :], xq[:, q, :],
                                           rs[:, :QW])
                    yv = y[128 * 8:128 * 9, :].rearrange(
                        "p (q c) -> p q c", q=4)
                    store_eng.dma_start(
                        yv[:, :, qt * QW:(qt + 1) * QW], xq[:],
                    )
                    if qt < 3:
                        reread(qt + 1)

                # interleave chunk 8's serialized quarters between stash
                # chunks so each re-read hides behind the previous stash
                # chunk's adds
                order = [0, 1, 2, 3, (8, 0), 4, (8, 1), 5, (8, 2), 6,
                         (8, 3), 7]
                for item in order:
                    if isinstance(item, tuple):
                        do_quarter8(item[1])
                        continue
                    r = item
                    buf = st[r]
                    L = make_lhsr(r)
                    xq = buf.rearrange("p (q c) -> p q c", q=4)
                    for h in range(2):
                        rs = res_half(L, h)
                        sl = slice(h * (HW0 // 2), (h + 1) * (HW0 // 2))
                        for q in range(4):
                            add_eng.tensor_add(xq[:, q, sl], xq[:, q, sl],
                                               rs[:])
                    store_eng.dma_start(y[128 * r:128 * (r + 1), :], buf[:])
    return nc


def _host_prep(x, gcn_masks, W_emb, W_gcn, b_gcn):
    x = np.asarray(x)
    gcn_masks = np.asarray(gcn_masks)
    wembT = np.asarray(W_emb).T.astype(BF_NP)
    wgcnv = np.ascontiguousarray(np.asarray(W_gcn)).astype(BF_NP)
    bbv = np.ascontiguousarray(
        np.broadcast_to(np.asarray(b_gcn, np.float32)[None, :], (N, C))
    )
    in_maps = []
    for b in range(B):
        # phase-major layout: [TC, dh, dw, h0, w0] so the 2x2 box-sum and
        # the nearest-upsample residual add are step-1 ops on device
        xb = np.ascontiguousarray(
            np.asarray(x[:, b]).reshape(T * C, H0, 2, W0, 2)
            .transpose(0, 2, 4, 1, 3).reshape(T * C, HW)
        ).astype(BF_NP)
        m = gcn_masks[b].reshape(T, K, HW0).astype(BF_NP)
        m56v = np.ascontiguousarray(m.reshape(N, HW0))
        mTpv = np.ascontiguousarray(
            m.reshape(T, K, NJ, JW).transpose(3, 0, 2, 1).reshape(JW, T * NJ * K)
        )
        in_maps.append({
            "x": xb, "m56": m56v, "mTp": mTpv,
            "wembT": wembT, "wgcn": wgcnv, "bb": bbv,
        })
    return in_maps


_NC_CACHE = {}


def kernel(x, gcn_masks, W_emb, W_gcn, b_gcn):
    from concourse.bass_utils import run_bass_kernel_spmd

    in_maps = _host_prep(x, gcn_masks, W_emb, W_gcn, b_gcn)
    if "nc" not in _NC_CACHE:
        _NC_CACHE["nc"] = build_nc(reps=1)
    nc = _NC_CACHE["nc"]
    res = run_bass_kernel_spmd(nc, in_maps, list(range(B)))
    out = np.empty((T, B, C, H, W), np.float32)
    for b in range(B):
        yb = res.results[b]["y"].astype(np.float32)
        out[:, b] = (
            yb.reshape(T * C, 2, 2, H0, W0).transpose(0, 3, 1, 4, 2)
            .reshape(T, C, H, W)
        )
    return out


# revision 59
# speedup vs baseline: 1.2487x; 1.2487x over previous
"""Trainium2 Bass kernel for nn_Encoder_90469191122997 (gnn_message_passing).

Data-parallel over batch B=8: core b owns batch b end-to-end.

v3: 16-bit traffic + SBUF residency + phase-major layout + no barriers.

x is sent to the device as bf16 (host cast) in PHASE-MAJOR layout
([TC, dh, dw, h0, w0], the four 2x2-nearest phases separated), y returns
bf16 phase-major (host upcast + re-interleave).  The 2e-2 rel-err gate
leaves ~4x margin.

Per core x_b = [T*C, HW] bf16 = 28.9 MB.  Chunks 0..6 (128 rows each)
stay resident in SBUF between the pooling pass and the residual pass;
chunks 7, 8 stream through two quarter-width buffers and are re-read in
pass 2 (re-read DMAs trickle in as slots free so DMA rarely idles).
HBM traffic/core: 28.9 (read) + 6.4 (re-read) + 28.9 (write) = 64 MB vs
173 MB for the fp32 two-pass baseline.

Phase-major makes the 2x2 box-sum three full-width step-1 bf16 adds
(DVE 2x packed mode) and the nearest-upsample residual adds pure step-1
adds against a PSUM->SBUF copy of the residual (also 2x).  Pooling
contracts the box-summed transpose (PE, 112-col grid, no tail) against
pre-transposed masks; the 18-node GCN runs on-chip in fp32 PSUM with
bf16 operands.  All tile pools live at one scope: no per-rep drain
barriers; x2 and m56 timeshare one buffer (m56 re-loaded per rep).
"""

import numpy as np
import ml_dtypes

import concourse.bass as bass
import concourse.mybir as mybir
import concourse.tile as tile
from concourse.masks import make_identity

T, B, C, H, W = 6, 8, 192, 112, 112
K = 3
H0, W0 = 56, 56
HW = H * W            # 12544
HW0 = H0 * W0         # 3136
QW = HW0 // 4         # 784, quarter width at 56-res (phase-plane cols)
N = T * K             # 18
CH = 96               # c half
NJ = 28               # pooling blocks per chunk (112-col grid, no tail)
JW = 112              # pooling block width
NR = 8                # residual blocks per chunk
RW = HW0 // NR        # 392, residual block width at 56-res
NCH = T * C // 128    # 9 row-chunks of 128 (t,c) rows each
NSTASH = 7            # chunks 0..6 resident in SBUF; 7, 8 streamed+re-read

BF = mybir.dt.bfloat16
F32 = mybir.dt.float32
BF_NP = ml_dtypes.bfloat16


def _spans(r):
    """(t, lo, hi, clo): rows [lo,hi) of chunk r belong to t, starting at
    channel clo.  Chunk boundaries hit t-edges only at offsets 0/64."""
    out = []
    for t in range(T):
        lo = max(128 * r, C * t)
        hi = min(128 * r + 128, C * (t + 1))
        if lo < hi:
            out.append((t, lo - 128 * r, hi - 128 * r, lo - C * t))
    return out


_LAST_CHUNK = {t: (C * (t + 1) - 1) // 128 for t in range(T)}

_MAX_WAITS = 1


def _split_multi_waits(nc):
    """This container's walrus rejects >1 sem wait per instruction ("Too many
    sync wait commands").  Move extra waits onto same-engine NoOps inserted
    immediately before the instruction (per-engine program order preserved)."""
    for bb in nc.main_func.blocks:
        insts = list(bb.instructions)
        if not any(
            i.sync_info and i.sync_info.on_wait
            and len(i.sync_info.on_wait) > _MAX_WAITS
            for i in insts
        ):
            continue
        new = []
        for inst in insts:
            si = inst.sync_info
            if si and si.on_wait and len(si.on_wait) > _MAX_WAITS:
                extra = list(si.on_wait[_MAX_WAITS:])
                del si.on_wait[_MAX_WAITS:]
                while extra:
                    chunk, extra = extra[:_MAX_WAITS], extra[_MAX_WAITS:]
                    nop = mybir.InstNoOp(
                        name=nc.get_next_instruction_name(),
                        engine=inst.engine,
                        bass_nofuse=True,
                        sync_info=mybir.SyncInfo(on_wait=chunk, on_update=[]),
                    )
                    nc.register_instruction(nop, overwrite=True)
                    new.append(nop)
            new.append(inst)
        bb.instructions = new


_orig_drain_and_barrier = tile.TileContext._drain_and_barrier


def _patched_drain_and_barrier(self, tick_clock, wait_clock):
    _orig_drain_and_barrier(self, tick_clock, wait_clock)
    _split_multi_waits(self.nc)


tile.TileContext._drain_and_barrier = _patched_drain_and_barrier


KNOBS = dict(
    copy_eng='scalar',     # engine for PSUM->SBUF transpose-tile copies
    add_eng='vector',      # engine for pass-2 residual adds
    rcopy_eng='scalar',    # engine for pass-2 res PSUM->SBUF copies
    store_eng='scalar',    # engine issuing y store DMAs
    x2T_bufs=3, tr_bufs=2, res_bufs=2, rsb_bufs=2, feat_bufs=2,
    group=2,               # j-blocks per x2T copy tile (1, 2 or 4)
    pool_chunks=(),        # stash chunks whose box-sum runs on POOL
)

if __name__ != "__main__":
    import json as _json
    import os as _os
    _ov = _os.environ.get("KERNEL_KNOBS")
    if _ov:
        KNOBS.update(_json.loads(_ov))


def _copy(eng, dst, src):
    if hasattr(eng, 'tensor_copy'):
        eng.tensor_copy(dst, src)
    else:
        eng.copy(dst, src)


def build_nc(reps: int = 1) -> bass.Bass:
    nc = bass.Bass()
    x = nc.dram_tensor("x", [T * C, HW], BF, kind="ExternalInput")
    m56 = nc.dram_tensor("m56", [N, HW0], BF, kind="ExternalInput")
    mTp = nc.dram_tensor("mTp", [JW, T * NJ * K], BF, kind="ExternalInput")
    wembT = nc.dram_tensor("wembT", [C, C], BF, kind="ExternalInput")
    wgcn = nc.dram_tensor("wgcn", [C, C], BF, kind="ExternalInput")
    bb = nc.dram_tensor("bb", [N, C], F32, kind="ExternalInput")
    y = nc.dram_tensor("y", [T * C, HW], BF, kind="ExternalOutput")

    copy_eng = getattr(nc, KNOBS['copy_eng'])
    add_eng = getattr(nc, KNOBS['add_eng'])
    rcopy_eng = getattr(nc, KNOBS['rcopy_eng'])
    store_eng = getattr(nc, KNOBS['store_eng'])
    GRP = KNOBS['group']
    XW = 128 * GRP

    with tile.TileContext(nc) as tc:
        with (
            tc.tile_pool(name="persist", bufs=1) as pp,
            tc.tile_pool(name="x2Tpool", bufs=KNOBS['x2T_bufs']) as x2Tpool,
            tc.tile_pool(name="smallsb", bufs=1) as ssb,
            tc.tile_pool(name="ressb", bufs=KNOBS['rsb_bufs']) as rsb,
            tc.tile_pool(name="lhsrp", bufs=2) as lhsrp,
            tc.tile_pool(name="trbps", bufs=KNOBS['tr_bufs'],
                         space="PSUM") as trbps,
            tc.tile_pool(name="featps", bufs=KNOBS['feat_bufs'],
                         space="PSUM") as fps,
            tc.tile_pool(name="ntps", bufs=1, space="PSUM") as ntps,
            tc.tile_pool(name="resps", bufs=KNOBS['res_bufs'],
                         space="PSUM") as rps,
            tc.tile_pool(name="gcnps", bufs=1, space="PSUM") as gps,
        ):
            ident = pp.tile([128, 128], BF)
            make_identity(nc, ident)
            ident32 = pp.tile([32, 32], F32, tag="ident32")
            make_identity(nc, ident32)
            mTp_sb = pp.tile([JW, T * NJ * K], BF)
            nc.sync.dma_start(mTp_sb[:], mTp[:])
            wemb_h = []
            wgcn_h = []
            for hh in range(2):
                wt = pp.tile([CH, C], BF, tag=f"wemb{hh}")
                nc.sync.dma_start(wt[:], wembT[hh * CH:(hh + 1) * CH, :])
                wemb_h.append(wt)
                gt = pp.tile([CH, C], BF, tag=f"wgcn{hh}")
                nc.sync.dma_start(gt[:], wgcn[hh * CH:(hh + 1) * CH, :])
                wgcn_h.append(gt)
            bb_sb = pp.tile([N, C], F32)
            nc.sync.dma_start(bb_sb[:], bb[:])

            # resident x chunks, two quarter-width stream buffers, and the
            # x2 / m56 timeshared scratch
            st = [
                pp.tile([128, HW], BF, tag=f"stash{i}", name=f"stash{i}")
                for i in range(NSTASH)
            ]
            hb = [
                pp.tile([128, HW // 4], BF, tag=f"hb{i}", name=f"hb{i}")
                for i in range(2)
            ]
            u = pp.tile([128, HW0], BF, tag="u", name="u")
            x2 = u            # pass-1 box-sum scratch (all 128 partitions)
            m56_sb = u        # pass-2 masks live in partitions 0..18

            for rep in range(reps):
                nodeT_h = [
                    pp.tile([CH, N], BF, tag=f"nodeT{hh}", name=f"nodeT{hh}")
                    for hh in range(2)
                ]
                outgb = pp.tile([N, C], BF, tag="outgb", name="outgb")

                # ---------------- pass 1: pooling ----------------
                feat_ps = {}

                def do_blocks(r, jj):
                    """x2T tile for 1-GRP j-blocks of chunk r + pooling
                    matmuls.  Grouping amortizes the PSUM->SBUF copy."""
                    tr = trbps.tile([JW, XW], BF, tag="trb")
                    for i, j in enumerate(jj):
                        nc.tensor.transpose(
                            tr[:, 128 * i:128 * (i + 1)],
                            x2[:, j * JW:(j + 1) * JW], ident[:],
                        )
                    x2T = x2Tpool.tile([JW, XW], BF, tag="x2T")
                    w = 128 * len(jj)
                    _copy(copy_eng, x2T[:, :w], tr[:, :w])
                    for i, j in enumerate(jj):
                        for (t, lo, hi, clo) in _spans(r):
                            col = (t * NJ + j) * K
                            nc.tensor.matmul(
                                feat_ps[t][:, clo:clo + (hi - lo)],
                                mTp_sb[:, col:col + K],
                                x2T[:, 128 * i + lo:128 * i + hi],
                                start=(j == 0), stop=(j == NJ - 1),
                                skip_group_check=True,
                            )

                def grouped(jlist):
                    for i in range(0, len(jlist), GRP):
                        yield tuple(jlist[i:i + GRP])

                def boxsum(xq, cols, eng):
                    """x2[:, cols] = sum of the 4 phase planes (step-1 bf16
                    adds, DVE 2x packed mode)."""
                    out = x2[:, cols]
                    eng.tensor_add(out, xq[:, 0, :], xq[:, 1, :])
                    eng.tensor_add(out, out, xq[:, 2, :])
                    eng.tensor_add(out, out, xq[:, 3, :])

                def close_feat(r):
                    for (t, lo, hi, clo) in _spans(r):
                        if _LAST_CHUNK[t] != r:
                            continue
                        feat_sb = ssb.tile([K, C], BF, tag="feat_sb")
                        nc.scalar.mul(feat_sb[:], feat_ps.pop(t)[:], 1.0 / HW)
                        for hh in range(2):
                            ntr = ntps.tile([CH, K], BF, tag="ntr")
                            nc.tensor.transpose(
                                ntr[:],
                                feat_sb[:, hh * CH:(hh + 1) * CH],
                                ident[:K, :K],
                            )
                            nc.any.tensor_copy(
                                nodeT_h[hh][:, K * t:K * (t + 1)], ntr[:]
                            )

                def open_feat(r):
                    for (t, lo, hi, clo) in _spans(r):
                        if t not in feat_ps:
                            feat_ps[t] = fps.tile(
                                [K, C], F32, tag="feat_ps", name=f"featps{t}"
                            )

                for r in range(NSTASH):
                    buf = st[r]
                    nc.sync.dma_start(buf[:], x[128 * r:128 * (r + 1), :])
                    xq = buf.rearrange("p (q c) -> p q c", q=4)
                    open_feat(r)
                    eng = (nc.gpsimd if r in KNOBS['pool_chunks']
                           else nc.vector)
                    # box-sum in two halves so PE transposes of the first
                    # half overlap DVE summing the second
                    for hf in range(2):
                        sl = slice(hf * (HW0 // 2), (hf + 1) * (HW0 // 2))
                        boxsum(xq[:, :, sl], sl, eng)
                        for jj in grouped(list(range(hf * (NJ // 2),
                                                     (hf + 1) * (NJ // 2)))):
                            do_blocks(r, jj)
                    close_feat(r)

                # chunks 7, 8 stream through the quarter buffers: quarter qt
                # of chunk r holds phase-plane cols [qt*QW, (qt+1)*QW)
                def qslice(r, qt):
                    xv = x[128 * r:128 * (r + 1), :].rearrange(
                        "p (q c) -> p q c", q=4)
                    return xv[:, :, qt * QW:(qt + 1) * QW]

                for r in (7, 8):
                    open_feat(r)
                    for qt in range(4):
                        buf = hb[qt % 2]
                        nc.sync.dma_start(
                            buf.rearrange("p (q c) -> p q c", q=4)[:],
                            qslice(r, qt),
                        )
                        sl = slice(qt * QW, (qt + 1) * QW)
                        boxsum(buf.rearrange("p (q c) -> p q c", q=4), sl,
                               nc.vector)
                        # a quarter holds 7 j-blocks: group within it
                        jq = list(range(qt * (NJ // 4), (qt + 1) * (NJ // 4)))
                        for jj in grouped(jq):
                            do_blocks(r, jj)
                    close_feat(r)

                # pass-2 data for chunks 7, 8 comes back via the same two
                # quarter buffers.  Issue the first two re-reads now (they
                # land during the GCN); the rest are issued in pass 2 as
                # each slot's previous quarter is consumed.
                QQ = [(r, qt) for r in (7, 8) for qt in range(4)]

                def reread(i):
                    r, qt = QQ[i]
                    nc.sync.dma_start(
                        hb[qt % 2].rearrange("p (q c) -> p q c", q=4)[:],
                        qslice(r, qt),
                    )

                reread(0)
                reread(1)
                # m56 shares u with x2: load after pass 1's last transpose
                nc.sync.dma_start(m56_sb[:N, :], m56[:])

                # ---------------- GCN on [18, 192] ----------------
                # one PSUM bank: every tile is a slice of the single 'g' tag
                def gtile():
                    return gps.tile([128, C], F32, tag="g", name="g")

                adjL = gtile()
                for hh in range(2):
                    nc.tensor.matmul(
                        adjL[:N, :N], nodeT_h[hh][:], nodeT_h[hh][:],
                        start=(hh == 0), stop=(hh == 1),
                    )
                mx = ssb.tile([N, 1], F32, tag="mx")
                nc.vector.reduce_max(mx[:], adjL[:N, :N],
                                     axis=mybir.AxisListType.X)
                nmx = ssb.tile([N, 1], F32, tag="nmx")
                nc.vector.tensor_scalar_mul(nmx[:], mx[:], -1.0)
                e_sb = ssb.tile([N, N], F32, tag="e_sb")
                nc.scalar.activation(
                    e_sb[:], adjL[:N, :N], mybir.ActivationFunctionType.Exp,
                    bias=nmx[:], scale=1.0,
                )
                s_ = ssb.tile([N, 1], F32, tag="s_")
                nc.vector.reduce_sum(s_[:], e_sb[:], axis=mybir.AxisListType.X)
                r_ = ssb.tile([N, 1], F32, tag="r_")
                nc.vector.reciprocal(r_[:], s_[:])
                adj_f = ssb.tile([N, N], F32, tag="adj_f")
                nc.vector.tensor_scalar_mul(adj_f[:], e_sb[:], r_[:])

                aaa_ps = gtile()
                for hh in range(2):
                    nc.tensor.matmul(
                        aaa_ps[:N, :C], nodeT_h[hh][:], wemb_h[hh][:],
                        start=(hh == 0), stop=(hh == 1),
                    )
                aaa_f = ssb.tile([N, C], F32, tag="aaa_f")
                nc.scalar.copy(aaa_f[:], aaa_ps[:N, :C])
                aaaT_h = []
                for hh in range(2):
                    aT_ps = gtile()
                    nc.tensor.transpose(
                        aT_ps[:CH, :N], aaa_f[:, hh * CH:(hh + 1) * CH],
                        ident32[:N, :N],
                    )
                    aT = ssb.tile([CH, N], BF, tag=f"aaaT{hh}")
                    nc.scalar.copy(aT[:], aT_ps[:CH, :N])
                    aaaT_h.append(aT)
                supp_ps = gtile()
                for hh in range(2):
                    nc.tensor.matmul(
                        supp_ps[:N, :C], aaaT_h[hh][:], wgcn_h[hh][:],
                        start=(hh == 0), stop=(hh == 1),
                    )
                supp_b = ssb.tile([N, C], BF, tag="supp_b")
                nc.scalar.copy(supp_b[:], supp_ps[:N, :C])
                adjT_ps = gtile()
                nc.tensor.transpose(adjT_ps[:N, :N], adj_f[:],
                                    ident32[:N, :N])
                adjT_b = ssb.tile([N, N], BF, tag="adjT_b")
                nc.scalar.copy(adjT_b[:], adjT_ps[:N, :N])
                outg_ps = gtile()
                nc.tensor.matmul(
                    outg_ps[:N, :C], adjT_b[:], supp_b[:], start=True,
                    stop=True,
                )
                nc.vector.tensor_add(outgb[:], outg_ps[:N, :C], bb_sb[:])

                # ---------------- pass 2: residual ----------------
                def make_lhsr(r):
                    """[18, 128] tile: outg rows 3t:3t+3 in the column range
                    of each t-span, zeros elsewhere.  Small SBUF->SBUF DMAs
                    go on the store queue to stay clear of the re-reads."""
                    L = lhsrp.tile([N, 128], BF, tag="lhsr")
                    nc.vector.memset(L[:], 0.0)
                    for (t, lo, hi, clo) in _spans(r):
                        store_eng.dma_start(
                            L[K * t:K * (t + 1), lo:hi],
                            outgb[K * t:K * (t + 1), clo:clo + (hi - lo)],
                        )
                    return L

                def res_half(L, h):
                    """Residual for hw0 cols [h*HW0/2, (h+1)*HW0/2) ->
                    bf16 SBUF tile (copies feed the 2x-mode adds)."""
                    rs = rsb.tile([128, HW0 // 2], BF, tag="rs")
                    for j in range(NR // 2):
                        res = rps.tile([128, RW], F32, tag="res")
                        nc.tensor.matmul(
                            res[:],
                            L[:],
                            m56_sb[:N, (h * NR // 2 + j) * RW:
                                   (h * NR // 2 + j + 1) * RW],
                            start=True, stop=True,
                        )
                        _copy(rcopy_eng, rs[:, j * RW:(j + 1) * RW], res[:])
                    return rs

                # streamed chunks first: their re-reads are already in flight
                for r in (7, 8):
                    L = make_lhsr(r)
                    for h in range(2):
                        rs = res_half(L, h)
                        for qt in (2 * h, 2 * h + 1):
                            buf = hb[qt % 2]
                            xq = buf.rearrange("p (q c) -> p q c", q=4)
                            rsl = rs[:, (qt % 2) * QW:(qt % 2 + 1) * QW]
                            for q in range(4):
                                add_eng.tensor_add(xq[:, q, :], xq[:, q, :],
                                                   rsl)
                            yv = y[128 * r:128 * (r + 1), :].rearrange(
                                "p (q c) -> p q c", q=4)
                            store_eng.dma_start(
                                yv[:, :, qt * QW:(qt + 1) * QW], xq[:],
                            )
                            nxt = QQ.index((r, qt)) + 2
                            if nxt < len(QQ):
                                reread(nxt)
                for r in range(NSTASH):
                    buf = st[r]
                    L = make_lhsr(r)
                    xq = buf.rearrange("p (q c) -> p q c", q=4)
                    for h in range(2):
                        rs = res_half(L, h)
                        sl = slice(h * (HW0 // 2), (h + 1) * (HW0 // 2))
                        for q in range(4):
                            add_eng.tensor_add(xq[:, q, sl], xq[:, q, sl],
                                               rs[:])
                    store_eng.dma_start(y[128 * r:128 * (r + 1), :], buf[:])
    return nc


def _host_prep(x, gcn_masks, W_emb, W_gcn, b_gcn):
    x = np.asarray(x)
    gcn_masks = np.asarray(gcn_masks)
    wembT = np.asarray(W_emb).T.astype(BF_NP)
    wgcnv = np.ascontiguousarray(np.asarray(W_gcn)).astype(BF_NP)
    bbv = np.ascontiguousarray(
        np.broadcast_to(np.asarray(b_gcn, np.float32)[None, :], (N, C))
    )
    in_maps = []
    for b in range(B):
        # phase-major layout: [TC, dh, dw, h0, w0] so the 2x2 box-sum and
        # the nearest-upsample residual add are step-1 ops on device
        xb = np.ascontiguousarray(
            np.asarray(x[:, b]).reshape(T * C, H0, 2, W0, 2)
            .transpose(0, 2, 4, 1, 3).reshape(T * C, HW)
        ).astype(BF_NP)
        m = gcn_masks[b].reshape(T, K, HW0).astype(BF_NP)
        m56v = np.ascontiguousarray(m.reshape(N, HW0))
        mTpv = np.ascontiguousarray(
            m.reshape(T, K, NJ, JW).transpose(3, 0, 2, 1).reshape(JW, T * NJ * K)
        )
        in_maps.append({
            "x": xb, "m56": m56v, "mTp": mTpv,
            "wembT": wembT, "wgcn": wgcnv, "bb": bbv,
        })
    return in_maps


_NC_CACHE = {}


def kernel(x, gcn_masks, W_emb, W_gcn, b_gcn):
    from concourse.bass_utils import run_bass_kernel_spmd

    in_maps = _host_prep(x, gcn_masks, W_emb, W_gcn, b_gcn)
    if "nc" not in _NC_CACHE:
        _NC_CACHE["nc"] = build_nc(reps=1)
    nc = _NC_CACHE["nc"]
    res = run_bass_kernel_spmd(nc, in_maps, list(range(B)))
    out = np.empty((T, B, C, H, W), np.float32)
    for b in range(B):
        yb = res.results[b]["y"].astype(np.float32)
        out[:, b] = (
            yb.reshape(T * C, 2, 2, H0, W0).transpose(0, 3, 1, 4, 2)
            .reshape(T, C, H, W)
        )
    return out
